# revision 36
# baseline (speedup 1.0000x reference)
"""Trainium2 Bass kernel for nn_DFlashSelfAttention (block-sparse GQA attention).

Self-contained: builds the Bass module once, shards inputs over 8 NeuronCores
(sequence-parallel), runs via run_bass_kernel_spmd, reassembles full output.
"""

import sys as _sys
for _p in ("/opt/trn_rl_repo",):
    if _p not in _sys.path:
        _sys.path.insert(0, _p)

"""Bass/Tile kernel for DFlashSelfAttention (block-diagonal causal attention).

Sharding: sequence-parallel over L (2048 -> 8 cores x 256 positions).
Attention is block-diagonal with BLOCK=16, so positions never interact
across 16-blocks; a 256-position slice (16 blocks) is fully independent.

Per-core pipeline (T = 512 rows = 2 batches x 256 positions), organized so
the PE streams dense GEMM work continuously while the attention softmax
chains ride in injected slots between GEMM groups:

  B : KV = X @ Wkv           (psum banks 0-3; 32 k-steps x 4 m-chunks)
  A1: Q[m0,m1] = X @ Wq      (banks 4-7)  + k-rope/krT injected
  A2: Q[m2,m3] = X @ Wq      (banks 0-3)  + attention(m0,m1) injected
  P5 half 0: Y[:, 0:256]     (banks 4-7)  + attention(m2,m3) injected
  P5 half 1: Y[:, 256:512]   (banks 4-7)  dense

Attention per (head pos, chunk):  S = krT.T @ qrT -> psum; est = exp(S)
(ACT, fp16; scores are bounded by 8 because q/k are RMS-normalized, so the
unmasked exp cannot overflow fp16); estm = est * mask01 (Pool); out.T+rowsum
via one matmul against V-augmented-with-ones ([q,65], lhsT=estm); recip (DVE)
of col 64; normalize (Pool, psum read) into pair tile; per-pair PE transpose
-> at_sb [feature, t] layout; ACT copies psum->at_sb.  This costs
128+65+64 PE rows per head vs 640 in a mask-preload/csum/broadcast scheme.

rstd for RMS-norm is exp(-0.5*ln(ms/HD+eps)) so every ACT op in the kernel
(ln, exp, copy, square) lives in one activation table set -> one table load.

All matmul operands are fp16 (1 cy/row); accumulation f32.  RMS-norm weights
and the sqrt(1/8) attention scale are folded into host-precomputed fp16 rope
tables.  Wq is SBUF-resident (streamed once); Wo mb0-3 resident, mb4-7
streamed per half.
"""

from collections import deque

import ml_dtypes
import numpy as np

import concourse.bass as bass
import concourse.mybir as mybir
import concourse.tile as tile
from concourse import bacc
from concourse.masks import make_identity

F32 = mybir.dt.float32
F16 = mybir.dt.float16

P = 128
HID = 4096
KO = HID // P          # 32 k-chunks over hidden
T = 512                # rows per core: 2 batches x 256 positions
NM = T // P            # 4 t-chunks
NH = 16
NKV = 4
HD = 64
QD = NH * HD           # 1024
KVD = 2 * NKV * HD     # 512 (k 256 | v 256)
EPS = 1e-6

# Q-head permutation: position p holds original head PERM[p]. Even positions
# carry heads whose KV head is even (partition half 0), odd positions heads
# with odd KV head (half 64) — so S-matmul operands share a base partition.
PERM = [0, 4, 1, 5, 2, 6, 3, 7, 8, 12, 9, 13, 10, 14, 11, 15]

EXP = mybir.ActivationFunctionType.Exp
LN = mybir.ActivationFunctionType.Ln
SQUARE = mybir.ActivationFunctionType.Square


def _combined_act_tables(arch):
    """Table list for the act-table-load pass with only the one set that
    covers every ACT function this kernel uses (ln/exp/square/copy) left
    non-empty.  The pass otherwise alternates exp_and_others <->
    natural_log on every stats16/exp interleave (1.3us per reload).
    Indices (= act_func_set_id) are preserved."""
    from concourse.hw_specs import get_activation_tables
    tabs = get_activation_tables(arch)
    need = {EXP, LN, SQUARE, mybir.ActivationFunctionType.Copy,
            mybir.ActivationFunctionType.Identity}
    chosen = None
    for name, s in tabs.items():
        if need.issubset(s):
            chosen = name
            break
    assert chosen is not None, "no activation table covers kernel funcs"
    return {name: (s if name == chosen else set())
            for name, s in tabs.items()}


def build_nc(name="dfa"):
    nc = bacc.Bacc(None, target_bir_lowering=False, name=name)
    arch = nc.m.arch
    bacc.get_activation_tables = lambda _arch: _combined_act_tables(arch)

    xt = nc.dram_tensor("xt", [HID, T], F16, kind="ExternalInput")
    wq = nc.dram_tensor("wq", [HID, QD], F16, kind="ExternalInput")
    wkv = nc.dram_tensor("wkv", [HID, KVD], F16, kind="ExternalInput")
    wo = nc.dram_tensor("wo", [32, P, 8, P], F16, kind="ExternalInput")
    cwq = nc.dram_tensor("cwq", [T, HD], F16, kind="ExternalInput")
    swq = nc.dram_tensor("swq", [T, HD], F16, kind="ExternalInput")
    cwk = nc.dram_tensor("cwk", [T, HD], F16, kind="ExternalInput")
    swk = nc.dram_tensor("swk", [T, HD], F16, kind="ExternalInput")
    mask = nc.dram_tensor("mask", [P, P], F16, kind="ExternalInput")
    yt = nc.dram_tensor("yt", [HID, T], F16, kind="ExternalOutput")

    from contextlib import ExitStack
    with tile.TileContext(nc) as tc, ExitStack() as ctx:
        consts = ctx.enter_context(tc.tile_pool(name="consts", bufs=1))
        xt_pool = ctx.enter_context(tc.tile_pool(name="xt", bufs=1))
        wq_pool = ctx.enter_context(tc.tile_pool(name="wq", bufs=1))
        wores = ctx.enter_context(tc.tile_pool(name="wores", bufs=1))
        wstream = ctx.enter_context(tc.tile_pool(name="wstream", bufs=3))
        acts = ctx.enter_context(tc.tile_pool(name="acts", bufs=1))
        rope_tmp = ctx.enter_context(tc.tile_pool(name="rope_tmp", bufs=1))
        tr_pool = ctx.enter_context(tc.tile_pool(name="tr", bufs=1))
        attn_tmp = ctx.enter_context(tc.tile_pool(name="attn_tmp", bufs=2))
        ystage = ctx.enter_context(tc.tile_pool(name="ystage", bufs=2))
        pp = ctx.enter_context(tc.tile_pool(name="pp", bufs=1, space="PSUM"))

        def ptile(shape, bank, name, dtype=F32):
            pad = 512 if dtype == F32 else 1024
            return pp.tile(shape, dtype, tag=f"b{bank}", name=name,
                           padded_shape=[P, pad])

        # ---- leading DMAs: small first tiles so PE starts fast ----
        xt_sb = xt_pool.tile([P, KO, T], F16)
        xt_r = xt.rearrange("(ko p) t -> p ko t", p=P)
        wkv_r = wkv.rearrange("(kb p) n -> p kb n", p=P)
        wq_r = wq.rearrange("(kb p) n -> p kb n", p=P)
        # wkv chunk list: two singles for fast start, then pairs
        wkv_chunks = [(0, 1), (1, 1)] + [(2 + 2 * i, 2) for i in range(15)]
        wkv_tiles = {}

        def wkv_dma(ci):
            k0, nk = wkv_chunks[ci]
            t_ = wstream.tile([P, nk, KVD], F16, tag="wkv", bufs=4,
                              name=f"wkv_c{ci}")
            nc.sync.dma_start(t_[:], wkv_r[:, k0:k0 + nk, :])
            for j in range(nk):
                wkv_tiles[k0 + j] = (t_, j)

        # identity first: Pool builds it while the first DMAs stream, so
        # the PE warmup transposes can start the p-state ramp early
        ident = consts.tile([P, P], F16)
        make_identity(nc, ident)
        eps_t = consts.tile([P, 1], F32)
        nc.vector.memset(eps_t, EPS)

        wkv_dma(0)
        nc.gpsimd.dma_start(xt_sb[:, 0:1, :], xt_r[:, 0:1, :])
        wkv_dma(1)
        nc.sync.dma_start(xt_sb[:, 1:2, :], xt_r[:, 1:2, :])
        wkv_dma(2)
        nc.sync.dma_start(xt_sb[:, 2:4, :], xt_r[:, 2:4, :])
        wkv_dma(3)
        wkv_dma(4)
        mask_sb = consts.tile([P, P], F16)
        nc.gpsimd.dma_start(mask_sb[:], mask[:])
        tabs = {}
        for nm_, dr_ in (("cwq", cwq), ("swq", swq), ("cwk", cwk),
                         ("swk", swk)):
            tt = consts.tile([P, NM, HD], F16, tag=nm_, name=nm_)
            nc.gpsimd.dma_start(tt[:], dr_.rearrange("(m p) d -> p m d", p=P))
            tabs[nm_] = tt

        # PE warmup: dead matmuls on a memset tile start the p-state ramp
        # while the first wkv/xt DMAs are still in flight.
        warm_src = consts.tile([P, 512], F16, tag="warm_src")
        nc.vector.memset(warm_src, 0.0)
        for w_ in range(8):
            wt_ = pp.tile([P, 512], F32, tag=f"b{4 + (w_ % 2)}",
                          name=f"warm{w_}", padded_shape=[P, 512])
            nc.tensor.matmul(wt_[:], warm_src[:, 0:P], warm_src[:])

        # V tiles augmented with a ones column: [P, g, 0:64]=V, [:, :, 64]=1
        vtil = []
        for m in range(NM):
            vt = acts.tile([P, NKV, HD + 1], F16, tag=f"v{m}", name=f"v{m}")
            nc.vector.memset(vt[:], 1.0)
            vtil.append(vt)

        wq_sb = wq_pool.tile([P, KO, QD], F16)

        # ---- phase B: KV = X @ Wkv (banks 0-3); xt/wkv/wq stream in-loop ----
        ps_b = [ptile([P, KVD], m, f"psb{m}") for m in range(NM)]
        next_wkv_ci = 5
        for k in range(KO):
            # prefetch the wkv chunk ~4 k ahead
            if next_wkv_ci < len(wkv_chunks) and                     wkv_chunks[next_wkv_ci][0] <= k + 5:
                wkv_dma(next_wkv_ci)
                next_wkv_ci += 1
            if k % 3 == 0 and 4 + (k // 3) * 4 < KO:
                xc = 4 + (k // 3) * 4
                nc.sync.dma_start(xt_sb[:, xc:xc + 4, :],
                                  xt_r[:, xc:xc + 4, :])
            if k >= 16 and k % 2 == 0:
                wc = k - 16  # wq chunks 0-15 during late B
                nc.sync.dma_start(wq_sb[:, wc:wc + 2, :],
                                  wq_r[:, wc:wc + 2, :])
            wt, j = wkv_tiles.pop(k)
            for m in range(NM):
                nc.tensor.matmul(
                    ps_b[m][:], xt_sb[:, k, m * P:(m + 1) * P],
                    wt[:, j, :],
                    start=(k == 0), stop=(k == KO - 1))

        # ---- B drain: kn16 + V copies ----
        kn16 = []
        for m in range(NM):
            kn = acts.tile([P, NKV, HD], F16, tag="kn", bufs=4,
                           name=f"kn{m}")
            nc.vector.tensor_copy(
                kn[:], ps_b[m][:, 0:256].rearrange("p (h d) -> p h d", d=HD))
            kn16.append(kn)
            nc.vector.tensor_copy(
                vtil[m][:, :, 0:HD],
                ps_b[m][:, 256:512].rearrange("p (h d) -> p h d", d=HD))

        def stats16(src_t, nh, tag):
            """x^2 sum -> rstd fp16 [P, nh] via exp(-0.5*ln(ms/HD+eps))."""
            sq = rope_tmp.tile([P, nh, HD], F32, tag=f"sq{nh}", name="sq",
                               bufs=1)
            nc.scalar.activation(sq[:], src_t[:], SQUARE)
            ms = rope_tmp.tile([P, nh], F32, tag=f"ms{nh}", bufs=1, name="ms")
            nc.vector.reduce_sum(ms[:], sq[:], axis=mybir.AxisListType.X)
            nc.scalar.activation(ms[:], ms[:], LN, bias=eps_t[:],
                                 scale=1.0 / HD)
            rstd = rope_tmp.tile([P, nh], F16, tag=f"rstd{nh}", bufs=4,
                                 name=tag)
            nc.scalar.activation(rstd[:], ms[:], EXP, scale=-0.5)
            return rstd

        def rope16(src_t, rstd, m, nh, ctab, stab, out_tag):
            """(src*rstd) rotary in fp16 -> fp16 tile [P, nh*HD]."""
            qn = rope_tmp.tile([P, nh, HD], F16, tag=f"qn{nh}", name="qn",
                               bufs=1)
            nc.vector.tensor_mul(qn[:], src_t[:],
                                 rstd[:, :, None].to_broadcast((P, nh, HD)))
            o1 = rope_tmp.tile([P, nh, HD], F16, tag=f"o1{nh}", name="o1",
                               bufs=1)
            nc.vector.tensor_mul(o1[:], qn[:],
                                 ctab[:, m, None, :].to_broadcast((P, nh, HD)))
            o2 = rope_tmp.tile([P, nh, HD], F16, tag=f"o2{nh}", name="o2",
                               bufs=1)
            H2 = HD // 2
            nc.vector.tensor_mul(
                o2[:, :, 0:H2], qn[:, :, H2:HD],
                stab[:, m, None, 0:H2].to_broadcast((P, nh, H2)))
            nc.vector.tensor_mul(
                o2[:, :, H2:HD], qn[:, :, 0:H2],
                stab[:, m, None, H2:HD].to_broadcast((P, nh, H2)))
            outt = rope_tmp.tile([P, nh * HD], F16, tag=out_tag,
                                 bufs=(4 if out_tag == "krout" else 2),
                                 name=out_tag)
            nc.vector.tensor_add(
                outt[:], o1[:].rearrange("p h d -> p (h d)"),
                o2[:].rearrange("p h d -> p (h d)"))
            return outt

        # ---- k-rope for all chunks (runs during A1) ----
        kr = []
        for m in range(NM):
            k_rstd = stats16(kn16[m], NKV, f"krstd{m}")
            kr.append(rope16(kn16[m], k_rstd, m, NKV, tabs["cwk"],
                             tabs["swk"], "krout"))

        krT = [[None, None] for _ in range(NM)]

        def krt_step(m, g2):
            bank = (2 * m + g2) % 4
            pt = ptile([P, P], bank, f"trk{m}_{g2}", F16)
            nc.tensor.matmul(pt[:], kr[m][:, g2 * P:(g2 + 1) * P],
                             ident[:], is_transpose=True)
            kt = tr_pool.tile([P, P], F16, tag=f"krT{m}_{g2}", bufs=1,
                              name=f"krT{m}_{g2}")
            nc.vector.tensor_copy(kt[:], pt[:])
            krT[m][g2] = kt

        # ---- phase A halves ----
        ps_a = {}

        WQ_A1A = {0: (16, 20), 1: (20, 24), 2: (24, 26), 3: (26, 28),
                  4: (28, 30), 5: (30, 32)}

        def a_chunk(m, banks2, inject, wq_stream=False):
            for s in range(2):
                ps_a[(m, s)] = ptile([P, 512], banks2[s], f"psa{m}_{s}")
            slot = 0
            for kb in range(8):
                if wq_stream and kb in WQ_A1A:
                    w0, w1 = WQ_A1A[kb]
                    nc.sync.dma_start(wq_sb[:, w0:w1, :], wq_r[:, w0:w1, :])
                for kk in range(4):
                    k = kb * 4 + kk
                    for s in range(2):
                        nc.tensor.matmul(
                            ps_a[(m, s)][:],
                            xt_sb[:, k, m * P:(m + 1) * P],
                            wq_sb[:, k, s * 512:(s + 1) * 512],
                            start=(k == 0), stop=(k == KO - 1))
                        inject(slot)
                    slot += 1

        def q_stats(m):
            qn_t = acts.tile([P, NH, HD], F16, tag="qnat", name=f"qnat{m}",
                             bufs=2)
            for s in range(2):
                nc.vector.tensor_copy(
                    qn_t[:, s * 8:(s + 1) * 8, :],
                    ps_a[(m, s)][:].rearrange("p (h d) -> p h d", d=HD))
            return qn_t, stats16(qn_t, NH, f"qrstd{m}")

        # ---- attention chain steps for one chunk ----
        at_sb = acts.tile([P, 8, T], F16, tag="at")

        def attn_steps(m, qr_get, sbanks, pvb, trb):
            """Returns list of closures: 8 qrT transposes + per-pos S/PV and
            per-pair transpose+copy, pipelined."""
            qrT = [None] * 8
            estm_t = {}
            ps_o = {}
            rt_t = {}
            apair = {}
            steps = []

            def t_step(hh):
                bank = sbanks[hh % 2]
                pt = ptile([P, P], bank, f"trq{m}_{hh}", F16)
                qr = qr_get()
                nc.tensor.matmul(pt[:], qr[:, hh * P:(hh + 1) * P],
                                 ident[:], is_transpose=True)
                qt = tr_pool.tile([P, P], F16, tag="qrT", bufs=16,
                                  name=f"qrT{m}_{hh}")
                nc.vector.tensor_copy(qt[:], pt[:])
                qrT[hh] = qt

            def a_step(pos):
                h = PERM[pos]
                g = h // 4
                base = (pos % 2) * HD
                hh = pos // 2
                sp = ptile([P, P], sbanks[pos % 2], f"s{m}_{pos}")
                nc.tensor.matmul(sp[:], krT[m][g // 2][base:base + HD, :],
                                 qrT[hh][base:base + HD, :],
                                 start=True, stop=True)
                est = attn_tmp.tile([P, P], F16, tag="est", bufs=4,
                                    name=f"est{m}_{pos}")
                nc.scalar.activation(est[:], sp[:], EXP)
                estm = attn_tmp.tile([P, P], F16, tag="estm", bufs=6,
                                     name=f"estm{m}_{pos}")
                nc.gpsimd.tensor_mul(estm[:], est[:], mask_sb[:])
                estm_t[pos] = estm

            def b_step(pos):
                h = PERM[pos]
                g = h // 4
                hh = pos // 2
                op = ptile([P, HD + 1], pvb, f"o{m}_{pos}")
                nc.tensor.matmul(op[:], estm_t.pop(pos)[:], vtil[m][:, g, :],
                                 start=True, stop=True)
                rt = attn_tmp.tile([P, 1], F32, tag="r", bufs=4,
                                   name=f"r{m}_{pos}")
                nc.vector.reciprocal(rt[:], op[:, HD:HD + 1])
                if hh not in apair:
                    apair[hh] = attn_tmp.tile([P, 2, HD], F16, tag="apair",
                                              bufs=3, name=f"ap{m}_{hh}")
                nc.scalar.activation(apair[hh][:, pos % 2, :], op[:, 0:HD],
                                     mybir.ActivationFunctionType.Copy,
                                     scale=rt[:])

            def c_step(hh):
                pt = ptile([P, P], trb, f"trc{m}_{hh}", F16)
                ap_f = apair.pop(hh)
                nc.tensor.matmul(pt[:], ap_f[:].rearrange("p s d -> p (s d)"),
                                 ident[:], is_transpose=True)
                nc.vector.tensor_copy(at_sb[:, hh, m * P:(m + 1) * P], pt[:])

            for hh in range(8):
                steps.append(lambda hh=hh: t_step(hh))
            for i in range(25):
                if i < 16:
                    steps.append(lambda i=i: a_step(i))
                if 6 <= i < 22:
                    steps.append(lambda i=i: b_step(i - 6))
                if i >= 10 and (i - 10) % 2 == 0 and (i - 10) // 2 < 8:
                    steps.append(lambda i=i: c_step((i - 10) // 2))
            return steps

        # ---- gated step scheduler across phases ----
        # phases: 0=A1a(m0) 1=A1b(m1) 2=A2a(m2) 3=A2b(m3) 4=P5h0 5=P5h1
        box = {}

        def q_rope_steps(m, key):
            """stats + rope for q chunk m as individual step closures, so
            the DVE work interleaves with chain ops instead of bursting."""
            st = {}

            def sq_f(s):
                if "sq" not in st:
                    st["sq"] = rope_tmp.tile([P, NH, HD], F32, tag="sq16",
                                             name="sq", bufs=1)
                nc.scalar.activation(
                    st["sq"][:, s * 8:(s + 1) * 8, :],
                    ps_a[(m, s)][:].rearrange("p (h d) -> p h d", d=HD),
                    SQUARE)

            def red_f():
                st["ms"] = rope_tmp.tile([P, NH], F32, tag="ms16", bufs=1,
                                         name="ms")
                nc.vector.reduce_sum(st["ms"][:], st["sq"][:],
                                     axis=mybir.AxisListType.X)

            def lnexp_f():
                nc.scalar.activation(st["ms"][:], st["ms"][:], LN,
                                     bias=eps_t[:], scale=1.0 / HD)
                st["rstd"] = rope_tmp.tile([P, NH], F16, tag="rstd16",
                                           bufs=4, name=f"qrstd{m}")
                nc.scalar.activation(st["rstd"][:], st["ms"][:], EXP,
                                     scale=-0.5)

            def mul_f(s):
                if "qnn" not in st:
                    st["qnn"] = rope_tmp.tile([P, NH, HD], F16, tag="qn16",
                                              name="qn", bufs=1)
                nc.vector.tensor_mul(
                    st["qnn"][:, s * 8:(s + 1) * 8, :],
                    ps_a[(m, s)][:].rearrange("p (h d) -> p h d", d=HD),
                    st["rstd"][:, s * 8:(s + 1) * 8, None]
                    .to_broadcast((P, 8, HD)))

            def o1_f():
                st["o1"] = rope_tmp.tile([P, NH, HD], F16, tag="o116",
                                         name="o1", bufs=1)
                nc.vector.tensor_mul(
                    st["o1"][:], st["qnn"][:],
                    tabs["cwq"][:, m, None, :].to_broadcast((P, NH, HD)))

            def o2_f(j):
                if "o2" not in st:
                    st["o2"] = rope_tmp.tile([P, NH, HD], F16, tag="o216",
                                             name="o2", bufs=1)
                H2 = HD // 2
                sl = (slice(0, H2), slice(H2, HD))[j]
                ssl = (slice(H2, HD), slice(0, H2))[j]
                nc.vector.tensor_mul(
                    st["o2"][:, :, sl], st["qnn"][:, :, ssl],
                    tabs["swq"][:, m, None, sl].to_broadcast((P, NH, H2)))

            def add_f():
                outt = rope_tmp.tile([P, NH * HD], F16, tag="qr", bufs=2,
                                     name="qr")
                nc.vector.tensor_add(
                    outt[:], st["o1"][:].rearrange("p h d -> p (h d)"),
                    st["o2"][:].rearrange("p h d -> p (h d)"))
                box[key] = outt

            return [lambda: sq_f(0), lambda: sq_f(1), red_f,
                    lnexp_f, lambda: mul_f(0), lambda: mul_f(1), o1_f,
                    lambda: o2_f(0), lambda: o2_f(1), add_f]

        class Sched:
            def __init__(self):
                self.q = deque()

            def add(self, gate, fn):
                self.q.append((gate, fn))

            def inject(self, phase, slot, n):
                for _ in range(n):
                    if not self.q:
                        return
                    gate, fn = self.q[0]
                    if (phase, slot) < gate:
                        return
                    self.q.popleft()
                    fn()

            def drain(self):
                while self.q:
                    self.q.popleft()[1]()

        sched = Sched()
        for i, (m_, g2_) in enumerate(
                (m_, g2_) for m_ in range(NM) for g2_ in range(2)):
            sched.add((0, 6 + 2 * i),
                      (lambda m_=m_, g2_=g2_: krt_step(m_, g2_)))
        for mm in range(4):
            gate_sr = (mm + 1, 0)
            gate_t = (mm + 1, 7) if mm == 3 else (mm + 1, 14)
            for st in q_rope_steps(mm, f"qr{mm}"):
                sched.add(gate_sr, st)
            s_m = attn_steps(mm, (lambda mm=mm: box[f"qr{mm}"]), (0, 1), 2, 3)
            for st in s_m:
                sched.add(gate_t, st)

        # ---- wo DMAs: mb0-3 resident, mb4-7 streamed ----
        wo_r = wo.rearrange("mo p ko j -> p mo ko j")
        wo_res = wores.tile([P, 16, 8, P], F16)  # mb 0-3
        wo_stream = {}

        def wo_dma(half, mb):
            if mb < 4:
                return
            for j in range(4):
                t_ = wstream.tile([P, 8, P], F16, tag="wo", bufs=6,
                                  name=f"wo_{half}_{mb}_{j}")
                mo = mb * 4 + j
                nc.sync.dma_start(t_[:], wo_r[:, mo, :, :])
                wo_stream[(half, mo)] = t_

        # ---- the four A phases ----
        a_chunk(0, (4, 5), lambda s: sched.inject(0, s, 1), wq_stream=True)
        a_chunk(1, (6, 7), lambda s: sched.inject(1, s, 1))
        # wo_res streams during A2a/A2b
        for h_ in range(2):
            nc.sync.dma_start(wo_res[:, h_ * 8:(h_ + 1) * 8, :, :],
                              wo_r[:, h_ * 8:(h_ + 1) * 8, :, :])

        def inject_a2a(s):
            sched.inject(2, s, 1)

        a_chunk(2, (4, 5), inject_a2a)

        def inject_a2b(s):
            if s == 0:
                wo_dma(0, 4)
            if s == 16:
                wo_dma(0, 5)
            sched.inject(3, s, 1)

        a_chunk(3, (6, 7), inject_a2b)

        yt_r = yt.rearrange("(mo p) t -> p mo t", p=P)

        def p5_half(half, inject=None):
            c0 = half * 256
            slot = 0
            mbs = list(range(8)) if half == 0 else [0, 4, 5, 6, 7, 1, 2, 3]
            for mi, mb in enumerate(mbs):
                split = (half == 1 and mi >= 6)
                ys = ystage.tile([P, 4, 256], F16, tag="ys", name="ys")
                for sub in range(4):
                    mo = mb * 4 + sub
                    if mb < 4:
                        wo_m = wo_res[:, mo, :, :]
                    else:
                        wo_m = wo_stream.pop((half, mo))[:]
                    if half == 0:
                        bank = 4 + (slot % 2 if slot < 4 else slot % 4)
                    else:
                        bank = slot % 8
                    ps = ptile([P, 256], bank, f"ps_y{half}_{mo}")
                    for k in range(8):
                        nc.tensor.matmul(ps[:], wo_m[:, k, :],
                                         at_sb[:, k, c0:c0 + 256],
                                         start=(k == 0), stop=(k == 7))
                    if sub % 2 == 0:
                        nc.scalar.copy(ys[:, sub, :], ps[:])
                    else:
                        nc.vector.tensor_copy(ys[:, sub, :], ps[:])
                    if split:
                        nc.sync.dma_start(
                            yt_r[:, mo:mo + 1, c0:c0 + 256],
                            ys[:, sub:sub + 1, :])
                    if inject is not None:
                        inject(slot)
                    slot += 1
                if not split:
                    nc.sync.dma_start(
                        yt_r[:, mb * 4:(mb + 1) * 4, c0:c0 + 256], ys[:])

        def inject_p5(slot):
            if slot == 0:
                wo_dma(0, 6)
            if slot == 4:
                wo_dma(0, 7)
            if slot == 8:
                wo_dma(1, 4)
            if slot == 12:
                wo_dma(1, 5)
            if slot == 16:
                wo_dma(1, 6)
            if slot == 20:
                wo_dma(1, 7)
            sched.inject(4, slot, 3)

        p5_half(0, inject_p5)
        sched.drain()
        p5_half(1)

    nc.finalize()
    return nc


def host_inputs(inputs, core):
    """Build the per-core DRAM input map from full problem inputs."""
    hs = np.asarray(inputs["hidden_states"], np.float32)
    am = np.asarray(inputs["attention_mask"], np.float32)
    cos = np.asarray(inputs["cos"], np.float32)
    sin = np.asarray(inputs["sin"], np.float32)
    Wqkv = np.asarray(inputs["Wqkv"], np.float32)
    Wo = np.asarray(inputs["Wo"], np.float32)
    qw = np.asarray(inputs["q_norm_w"], np.float32)
    kw = np.asarray(inputs["k_norm_w"], np.float32)

    LS = 256
    ls = slice(core * LS, (core + 1) * LS)
    X = hs[:, ls, :].reshape(T, HID)
    xt = np.ascontiguousarray(X.T).astype(np.float16)
    cos_c = cos[:, ls, :].reshape(T, HD)
    sin_c = sin[:, ls, :].reshape(T, HD)
    sq = float(HD) ** -0.25  # sqrt(1/sqrt(HD)) = sqrt(1/8)
    swap = np.concatenate([np.arange(32, 64), np.arange(0, 32)])
    sign = np.concatenate([-np.ones(32, np.float32), np.ones(32, np.float32)])
    m = {
        "xt": xt,
        "cwq": np.ascontiguousarray(cos_c * qw[None, :] * sq).astype(np.float16),
        "swq": np.ascontiguousarray(
            sin_c * qw[swap][None, :] * sign[None, :] * sq).astype(np.float16),
        "cwk": np.ascontiguousarray(cos_c * kw[None, :] * sq).astype(np.float16),
        "swk": np.ascontiguousarray(
            sin_c * kw[swap][None, :] * sign[None, :] * sq).astype(np.float16),
        "wq": np.ascontiguousarray(
            Wqkv[:, :QD].reshape(HID, NH, HD)[:, PERM, :]
            .reshape(HID, QD)).astype(np.float16),
        "wkv": np.ascontiguousarray(Wqkv[:, QD:]).astype(np.float16),
        "wo": np.ascontiguousarray(
            Wo.reshape(NH, HD, HID)[PERM].reshape(QD, HID)
              .reshape(8, P, 32, P).transpose(2, 1, 0, 3)).astype(np.float16),
        # 0/1 multiplicative mask, [key, query] orientation
        "mask": (am[0, 0, :P, :P].T >= -0.5).astype(np.float16),
    }
    return m


def assemble_output(yts):
    """yts: list of 8 [4096, 512] fp16 arrays -> [2, 2048, 4096] f32."""
    out = np.empty((2, 2048, HID), np.float32)
    for c, yt_ in enumerate(yts):
        sl = yt_.astype(np.float32).T.reshape(2, 256, HID)
        out[:, c * 256:(c + 1) * 256, :] = sl
    return out


_NC_CACHE = {}


def _get_nc():
    if "nc" not in _NC_CACHE:
        _NC_CACHE["nc"] = build_nc()
    return _NC_CACHE["nc"]


def _run(inputs, trace=False):
    from concourse.bass_utils import run_bass_kernel_spmd
    nc = _get_nc()
    in_maps = [host_inputs(inputs, c) for c in range(8)]
    res = run_bass_kernel_spmd(nc, in_maps, core_ids=list(range(8)),
                               trace=trace)
    out = assemble_output([res.results[c]["yt"] for c in range(8)])
    return out, res


def kernel(**inputs):
    out, _ = _run(inputs, trace=False)
    return out


def _timed_runs(inputs, n=20):
    """Amortized per-execution wall time (ns) of the compiled SPMD body with
    device-resident inputs. Used by test.py; not part of the grading path."""
    import time
    import jax
    from jax.sharding import Mesh, PartitionSpec, NamedSharding
    from jax.experimental.shard_map import shard_map
    import concourse.bass2jax as b2j
    import concourse.mybir as _mb

    nc = _get_nc()
    in_maps = [host_inputs(inputs, c) for c in range(8)]
    n_cores = 8
    b2j.install_neuronx_cc_hook()
    pname = nc.partition_id_tensor.name if nc.partition_id_tensor else None
    in_names, out_names, out_avals, zero_outs = [], [], [], []
    for alloc in nc.m.functions[0].allocations:
        if not isinstance(alloc, _mb.MemoryLocationSet):
            continue
        name = alloc.memorylocations[0].name
        if alloc.kind == "ExternalInput":
            if name != pname:
                in_names.append(name)
        elif alloc.kind == "ExternalOutput":
            out_names.append(name)
            shape = tuple(alloc.tensor_shape)
            dtype = _mb.dt.np(alloc.dtype)
            out_avals.append(jax.core.ShapedArray(shape, dtype))
            zero_outs.append(np.zeros(shape, dtype))
    n_params = len(in_names)
    all_in = list(in_names) + list(out_names)
    if pname is not None:
        all_in.append(pname)

    def _body(*args):
        operands = list(args)
        if pname is not None:
            operands.append(b2j.partition_id_tensor())
        return tuple(b2j._bass_exec_p.bind(
            *operands, out_avals=tuple(out_avals), in_names=tuple(all_in),
            out_names=tuple(out_names), lowering_input_output_aliases=(),
            sim_require_finite=True, sim_require_nnan=True, nc=nc))

    devices = jax.devices()[:n_cores]
    mesh = Mesh(np.asarray(devices), ("core",))
    specs = (PartitionSpec("core"),) * (n_params + len(out_names))
    fn = jax.jit(shard_map(_body, mesh=mesh, in_specs=specs,
                           out_specs=(PartitionSpec("core"),) * len(out_names),
                           check_rep=False), keep_unused=True)
    per_core = [[np.asarray(m[nm]) for nm in in_names] for m in in_maps]
    concat_in = [np.concatenate([per_core[c][i] for c in range(n_cores)])
                 for i in range(n_params)]
    concat_zero = [np.zeros((n_cores * z.shape[0], *z.shape[1:]), z.dtype)
                   for z in zero_outs]
    sh = NamedSharding(mesh, PartitionSpec("core"))
    dev_in = [jax.device_put(a, sh) for a in concat_in + concat_zero]
    out = fn(*dev_in)
    jax.block_until_ready(out)
    best = None
    for _ in range(3):
        t0 = time.time()
        for _ in range(n):
            out = fn(*dev_in)
        jax.block_until_ready(out)
        dt = (time.time() - t0) / n * 1e9
        best = dt if best is None else min(best, dt)
    return best
